# revision 11
# baseline (speedup 1.0000x reference)
"""RNN-T JointNetwork kernel for 8x Trainium2 NeuronCores.

Sharding: data-parallel over batch (B=8 -> 1 batch element per core).
Each core computes its (T, U, V) logit block fully on-chip.

v2 layout (u-major): the joint add+tanh is FUSED into single Activation
instructions using the per-partition bias operand:
    jt[j, u, :] = tanh(encP[j, :] * 1 + predP[j, u])
so the vector engine no longer does the broadcast add (it was the
co-bottleneck at ~68us); it only drains PSUM->SBUF casts, with a slice
of drains moved to the scalar engine to balance. Positions are ordered
(u, t) so each tanh covers a full T row; the host transposes back.

b_out is added on the host during the gather.
"""

import numpy as np
import ml_dtypes

P = 128
B, T, U = 8, 200, 50
DE, DP, DJ, V = 512, 640, 640, 1024
NDE, NDP, NJC, NVC = DE // P, DP // P, DJ // P, V // P  # 4, 5, 5, 8
UB = 2               # u's per psum block
PBLK = UB * T        # 400 joint positions per psum tile
NUB = U // UB        # 25 u-groups
VQ = 4               # v-chunks ganged per SBUF out tile
NVH = NVC // VQ      # 2 v-halves

BF16 = ml_dtypes.bfloat16

_module = None


def _build_module():
    import concourse.bass as bass
    import concourse.mybir as mybir
    import concourse.tile as tile
    from concourse import bacc

    bf = mybir.dt.bfloat16
    f32 = mybir.dt.float32
    Alu = mybir.AluOpType
    Act = mybir.ActivationFunctionType
    ts, ds = bass.ts, bass.ds

    nc = bacc.Bacc("TRN2", target_bir_lowering=False, debug=False)

    d_encT = nc.dram_tensor("encT", (P, NDE, T), bf, kind="ExternalInput").ap()
    d_predT = nc.dram_tensor("predT", (P, NDP, U), bf, kind="ExternalInput").ap()
    d_wenc8 = nc.dram_tensor("wenc8", (P, NJC, NDE, P), bf, kind="ExternalInput").ap()
    d_wpred8 = nc.dram_tensor("wpred8", (P, NJC, NDP, P), bf, kind="ExternalInput").ap()
    d_wout8 = nc.dram_tensor("wout8", (P, NVC, NJC, P), bf, kind="ExternalInput").ap()
    d_bj = nc.dram_tensor("bj", (P, NJC), f32, kind="ExternalInput").ap()
    d_out = nc.dram_tensor("out", (V, U * T), bf, kind="ExternalOutput").ap()

    with tile.TileContext(nc) as tc:
        with (
            tc.tile_pool(name="consts", bufs=1) as consts,
            tc.tile_pool(name="joints", bufs=15) as joints,
            tc.tile_pool(name="outsb", bufs=6) as outsb,
            tc.tile_pool(name="ps", bufs=8, space="PSUM") as pspool,
        ):
            # Input DMAs chunked per jc/vc in compute-priority order so the
            # projections chase the DMA stream (input DMA is ~9us at full
            # rate; all-at-once transfers stall the first matmul to ~17us).
            # Projection path on the sync ring; wout chunks on the (idle)
            # gpsimd ring so the scalar engine stays free for tanh.
            encT = consts.tile([P, NDE, T], bf)
            nc.sync.dma_start(encT[:], d_encT[:])
            predT = consts.tile([P, NDP, U], bf)
            nc.sync.dma_start(predT[:], d_predT[:])
            bj = consts.tile([P, NJC], f32)
            nc.sync.dma_start(bj[:], d_bj[:])
            wencs, wpreds, wouts = [], [], []
            for jc in range(NJC):
                we = consts.tile([P, NDE, P], bf, tag=f"we{jc}")
                nc.sync.dma_start(we[:], d_wenc8[:, jc])
                wencs.append(we)
                wp = consts.tile([P, NDP, P], bf, tag=f"wp{jc}")
                nc.sync.dma_start(wp[:], d_wpred8[:, jc])
                wpreds.append(wp)
            for vc in range(NVC):
                wo = consts.tile([P, NJC, P], bf, tag=f"wo{vc}")
                nc.gpsimd.dma_start(wo[:], d_wout8[:, vc])
                wouts.append(wo)

            # --- projections -> encP[j, t] f32, predP[j, u] f32 with
            # (b_enc+b_pred) folded in.
            encP = consts.tile([P, NJC, T], f32)
            predP = consts.tile([P, NJC, U], f32)
            for jc in range(NJC):
                ps_e = pspool.tile([P, 512], f32, tag="ps")
                for dc in range(NDE):
                    nc.tensor.matmul(
                        ps_e[:, :T], wencs[jc][:, dc, :], encT[:, dc, :],
                        start=(dc == 0), stop=(dc == NDE - 1),
                    )
                nc.vector.tensor_copy(encP[:, jc, :], ps_e[:, :T])

                ps_p = pspool.tile([P, 512], f32, tag="ps")
                for dc in range(NDP):
                    nc.tensor.matmul(
                        ps_p[:, :U], wpreds[jc][:, dc, :], predT[:, dc, :],
                        start=(dc == 0), stop=(dc == NDP - 1),
                    )
                nc.vector.tensor_tensor(
                    predP[:, jc, :], ps_p[:, :U],
                    bj[:, jc, None].to_broadcast((P, U)), Alu.add,
                )

            # --- main loop over u-groups (UB u's x full T per group)
            for ug in range(NUB):
                jtiles = []
                for jc in range(NJC):
                    jt = joints.tile([P, UB, T], bf, tag="jt")
                    for r in range(UB):
                        u = ug * UB + r
                        nc.scalar.activation(
                            jt[:, r, :], encP[:, jc, :], Act.Tanh,
                            bias=predP[:, jc, u, None],
                        )
                    jtiles.append(jt[:].rearrange("p a b -> p (a b)"))

                last = ug == NUB - 1
                for vh in range(NVH):
                    osb = outsb.tile([P, VQ, PBLK], bf, tag="osb")
                    for vq in range(VQ):
                        ps_o = pspool.tile([P, 512], f32, tag="ps")
                        for jc in range(NJC):
                            nc.tensor.matmul(
                                ps_o[:, :PBLK], wouts[vh * VQ + vq][:, jc, :],
                                jtiles[jc],
                                start=(jc == 0), stop=(jc == NJC - 1),
                            )
                        # drain split: one slice per group on the scalar
                        # engine, the rest on vector. For the final group
                        # alternate engines so the tail drains in parallel.
                        on_scalar = (vq & 1) if last else (vh == 0 and vq == 3)
                        if on_scalar:
                            nc.scalar.copy(osb[:, vq, :], ps_o[:, :PBLK])
                        else:
                            nc.vector.tensor_copy(osb[:, vq, :], ps_o[:, :PBLK])
                    dst = (
                        d_out[ds(vh * VQ * P, VQ * P), ts(ug, PBLK)]
                        .rearrange("(q p) c -> p q c", p=P)
                    )
                    if last:
                        # split the final DMA across two rings to cut the tail
                        nc.sync.dma_start(dst[:, :2, :], osb[:, :2, :])
                        nc.scalar.dma_start(dst[:, 2:, :], osb[:, 2:, :])
                    else:
                        nc.sync.dma_start(dst, osb[:])

    nc.compile()
    return nc


def _get_module():
    global _module
    if _module is None:
        _module = _build_module()
    return _module


def _chunk(x2d, dtype=BF16):
    """(n*128, C...) -> (128, n, C...) partition-chunked, contiguous."""
    n = x2d.shape[0] // P
    return np.ascontiguousarray(
        x2d.reshape((n, P) + x2d.shape[1:]).swapaxes(0, 1)
    ).astype(dtype)


def _wchunk(W, n_out, n_in):
    """nn.Linear weight (out, in) -> (128_in, n_out, n_in, 128_out):
    [p, oc, ic, q] = W[oc*128+q, ic*128+p], per-jc/vc chunks contiguous."""
    A = W.reshape(n_out, P, n_in, P)        # (oc, q, ic, p)
    return np.ascontiguousarray(A.transpose(3, 0, 2, 1)).astype(BF16)


def make_in_maps(encoder_out, predictor_out, W_enc, b_enc, W_pred, b_pred, W_out, b_out):
    wenc8 = _wchunk(W_enc, NJC, NDE)        # (128, 5, 4, 128)
    wpred8 = _wchunk(W_pred, NJC, NDP)      # (128, 5, 5, 128)
    wout8 = _wchunk(W_out, NVC, NJC)        # (128, 8, 5, 128)
    bj = np.ascontiguousarray(
        (b_enc + b_pred).reshape(NJC, P).T).astype(np.float32)   # (128, 5)
    in_maps = []
    for b in range(B):
        in_maps.append({
            "encT": _chunk(np.ascontiguousarray(encoder_out[b].T)),    # (128,4,200)
            "predT": _chunk(np.ascontiguousarray(predictor_out[b].T)), # (128,5,50)
            "wenc8": wenc8,
            "wpred8": wpred8,
            "wout8": wout8,
            "bj": bj,
        })
    return in_maps


def _postprocess(out_vt, b_out):
    """(V, U*T) device output (bf16, pos=(u,t)) -> (T, U, V) fp32 + bias."""
    arr = out_vt.astype(np.float32).T.reshape(U, T, V).swapaxes(0, 1)
    return arr + b_out.astype(np.float32)


def kernel(encoder_out, predictor_out, W_enc, b_enc, W_pred, b_pred, W_out, b_out):
    from concourse.bass_utils import run_bass_kernel_spmd

    nc = _get_module()
    in_maps = make_in_maps(
        encoder_out, predictor_out, W_enc, b_enc, W_pred, b_pred, W_out, b_out
    )
    res = run_bass_kernel_spmd(nc, in_maps, list(range(B)))
    out = np.empty((B, T, U, V), np.float32)
    for b in range(B):
        out[b] = _postprocess(res.results[b]["out"], b_out)
    return out


# revision 16
# speedup vs baseline: 1.1741x; 1.1741x over previous
"""RNN-T JointNetwork kernel for 8x Trainium2 NeuronCores.

Sharding: data-parallel over batch (B=8 -> 1 batch element per core).
Each core computes its (T, U, V) logit block fully on-chip.

v2 layout (u-major): the joint add+tanh is FUSED into single Activation
instructions using the per-partition bias operand:
    jt[j, u, :] = tanh(encP[j, :] * 1 + predP[j, u])
so the vector engine no longer does the broadcast add (it was the
co-bottleneck at ~68us); it only drains PSUM->SBUF casts, with a slice
of drains moved to the scalar engine to balance. Positions are ordered
(u, t) so each tanh covers a full T row; the host transposes back.

b_out is added on the host during the gather.
"""

import numpy as np
import ml_dtypes

P = 128
B, T, U = 8, 200, 50
DE, DP, DJ, V = 512, 640, 640, 1024
NDE, NDP, NJC, NVC = DE // P, DP // P, DJ // P, V // P  # 4, 5, 5, 8
UB = 2               # u's per psum block
PBLK = UB * T        # 400 joint positions per psum tile
NUB = U // UB        # 25 u-groups
VQ = 4               # v-chunks ganged per SBUF out tile
NVH = NVC // VQ      # 2 v-halves

BF16 = ml_dtypes.bfloat16

_module = None


def _build_module():
    import concourse.bass as bass
    import concourse.mybir as mybir
    import concourse.tile as tile
    from concourse import bacc

    bf = mybir.dt.bfloat16
    f32 = mybir.dt.float32
    Alu = mybir.AluOpType
    Act = mybir.ActivationFunctionType
    ts, ds = bass.ts, bass.ds

    nc = bacc.Bacc("TRN2", target_bir_lowering=False, debug=False)

    d_encT = nc.dram_tensor("encT", (P, NDE, T), bf, kind="ExternalInput").ap()
    d_predT = nc.dram_tensor("predT", (P, NDP, U), bf, kind="ExternalInput").ap()
    d_wenc8 = nc.dram_tensor("wenc8", (P, NJC, NDE, P), bf, kind="ExternalInput").ap()
    d_wpred8 = nc.dram_tensor("wpred8", (P, NJC, NDP, P), bf, kind="ExternalInput").ap()
    d_woutT = nc.dram_tensor("woutT", (P, NJC, V), bf, kind="ExternalInput").ap()
    d_bj = nc.dram_tensor("bj", (P, NJC), f32, kind="ExternalInput").ap()
    d_out = nc.dram_tensor("out", (V, U * T), bf, kind="ExternalOutput").ap()

    with tile.TileContext(nc) as tc:
        with (
            tc.tile_pool(name="consts", bufs=1) as consts,
            tc.tile_pool(name="joints", bufs=15) as joints,
            tc.tile_pool(name="outsb", bufs=6) as outsb,
            tc.tile_pool(name="ps", bufs=8, space="PSUM") as pspool,
        ):
            # Input DMAs chunked per jc/vc in compute-priority order so the
            # projections chase the DMA stream (input DMA is ~9us at full
            # rate; all-at-once transfers stall the first matmul to ~17us).
            # Projection path on the sync ring; wout chunks on the (idle)
            # gpsimd ring so the scalar engine stays free for tanh.
            encT = consts.tile([P, NDE, T], bf)
            nc.sync.dma_start(encT[:], d_encT[:])
            predT = consts.tile([P, NDP, U], bf)
            nc.sync.dma_start(predT[:], d_predT[:])
            bj = consts.tile([P, NJC], f32)
            nc.sync.dma_start(bj[:], d_bj[:])
            wencs, wpreds = [], []
            for jc in range(NJC):
                we = consts.tile([P, NDE, P], bf, tag=f"we{jc}")
                nc.sync.dma_start(we[:], d_wenc8[:, jc])
                wencs.append(we)
                wp = consts.tile([P, NDP, P], bf, tag=f"wp{jc}")
                nc.sync.dma_start(wp[:], d_wpred8[:, jc])
                wpreds.append(wp)
            # wout stays one big tile: splitting it into 8 small stationary
            # tiles measurably slowed every out-proj matmul (~36ns each).
            wout = consts.tile([P, NJC, V], bf)
            nc.sync.dma_start(wout[:], d_woutT[:])

            # --- projections -> encP[j, t] f32, predP[j, u] f32 with
            # (b_enc+b_pred) folded in.
            encP = consts.tile([P, NJC, T], f32)
            predP = consts.tile([P, NJC, U], f32)
            for jc in range(NJC):
                ps_e = pspool.tile([P, 512], f32, tag="ps")
                for dc in range(NDE):
                    nc.tensor.matmul(
                        ps_e[:, :T], wencs[jc][:, dc, :], encT[:, dc, :],
                        start=(dc == 0), stop=(dc == NDE - 1),
                    )
                nc.vector.tensor_copy(encP[:, jc, :], ps_e[:, :T])

                ps_p = pspool.tile([P, 512], f32, tag="ps")
                for dc in range(NDP):
                    nc.tensor.matmul(
                        ps_p[:, :U], wpreds[jc][:, dc, :], predT[:, dc, :],
                        start=(dc == 0), stop=(dc == NDP - 1),
                    )
                nc.vector.tensor_tensor(
                    predP[:, jc, :], ps_p[:, :U],
                    bj[:, jc, None].to_broadcast((P, U)), Alu.add,
                )

            # --- main loop over u-groups (UB u's x full T per group)
            for ug in range(NUB):
                jtiles = []
                for jc in range(NJC):
                    jt = joints.tile([P, UB, T], bf, tag="jt")
                    for r in range(UB):
                        u = ug * UB + r
                        nc.scalar.activation(
                            jt[:, r, :], encP[:, jc, :], Act.Tanh,
                            bias=predP[:, jc, u, None],
                        )
                    jtiles.append(jt[:].rearrange("p a b -> p (a b)"))

                last = ug == NUB - 1
                for vh in range(NVH):
                    osb = outsb.tile([P, VQ, PBLK], bf, tag="osb")
                    for vq in range(VQ):
                        ps_o = pspool.tile([P, 512], f32, tag="ps")
                        for jc in range(NJC):
                            nc.tensor.matmul(
                                ps_o[:, :PBLK], wout[:, jc, ts(vh * VQ + vq, P)],
                                jtiles[jc],
                                start=(jc == 0), stop=(jc == NJC - 1),
                            )
                        # drains on vector (scalar stays tanh-only); for the
                        # final group alternate engines so the tail drains
                        # in parallel.
                        on_scalar = (vq & 1) if last else False
                        if on_scalar:
                            nc.scalar.copy(osb[:, vq, :], ps_o[:, :PBLK])
                        else:
                            nc.vector.tensor_copy(osb[:, vq, :], ps_o[:, :PBLK])
                    dst = (
                        d_out[ds(vh * VQ * P, VQ * P), ts(ug, PBLK)]
                        .rearrange("(q p) c -> p q c", p=P)
                    )
                    if last:
                        # split the final DMA across two rings to cut the tail
                        nc.sync.dma_start(dst[:, :2, :], osb[:, :2, :])
                        nc.scalar.dma_start(dst[:, 2:, :], osb[:, 2:, :])
                    else:
                        nc.sync.dma_start(dst, osb[:])

    nc.compile()
    return nc


def _get_module():
    global _module
    if _module is None:
        _module = _build_module()
    return _module


def _chunk(x2d, dtype=BF16):
    """(n*128, C...) -> (128, n, C...) partition-chunked, contiguous."""
    n = x2d.shape[0] // P
    return np.ascontiguousarray(
        x2d.reshape((n, P) + x2d.shape[1:]).swapaxes(0, 1)
    ).astype(dtype)


def _wchunk(W, n_out, n_in):
    """nn.Linear weight (out, in) -> (128_in, n_out, n_in, 128_out):
    [p, oc, ic, q] = W[oc*128+q, ic*128+p], per-jc/vc chunks contiguous."""
    A = W.reshape(n_out, P, n_in, P)        # (oc, q, ic, p)
    return np.ascontiguousarray(A.transpose(3, 0, 2, 1)).astype(BF16)


def make_in_maps(encoder_out, predictor_out, W_enc, b_enc, W_pred, b_pred, W_out, b_out):
    wenc8 = _wchunk(W_enc, NJC, NDE)        # (128, 5, 4, 128)
    wpred8 = _wchunk(W_pred, NJC, NDP)      # (128, 5, 5, 128)
    woutT = _chunk(np.ascontiguousarray(W_out.T))       # (128, 5, 1024)
    bj = np.ascontiguousarray(
        (b_enc + b_pred).reshape(NJC, P).T).astype(np.float32)   # (128, 5)
    in_maps = []
    for b in range(B):
        in_maps.append({
            "encT": _chunk(np.ascontiguousarray(encoder_out[b].T)),    # (128,4,200)
            "predT": _chunk(np.ascontiguousarray(predictor_out[b].T)), # (128,5,50)
            "wenc8": wenc8,
            "wpred8": wpred8,
            "woutT": woutT,
            "bj": bj,
        })
    return in_maps


def _postprocess(out_vt, b_out):
    """(V, U*T) device output (bf16, pos=(u,t)) -> (T, U, V) fp32 + bias."""
    arr = out_vt.astype(np.float32).T.reshape(U, T, V).swapaxes(0, 1)
    return arr + b_out.astype(np.float32)


def kernel(encoder_out, predictor_out, W_enc, b_enc, W_pred, b_pred, W_out, b_out):
    from concourse.bass_utils import run_bass_kernel_spmd

    nc = _get_module()
    in_maps = make_in_maps(
        encoder_out, predictor_out, W_enc, b_enc, W_pred, b_pred, W_out, b_out
    )
    res = run_bass_kernel_spmd(nc, in_maps, list(range(B)))
    out = np.empty((B, T, U, V), np.float32)
    for b in range(B):
        out[b] = _postprocess(res.results[b]["out"], b_out)
    return out


# revision 20
# speedup vs baseline: 1.2003x; 1.0223x over previous
"""RNN-T JointNetwork kernel for 8x Trainium2 NeuronCores.

Sharding: data-parallel over batch (B=8 -> 1 batch element per core).
Each core computes its (T, U, V) logit block fully on-chip.

v2 layout (u-major): the joint add+tanh is FUSED into single Activation
instructions using the per-partition bias operand:
    jt[j, u, :] = tanh(encP[j, :] * 1 + predP[j, u])
so the vector engine no longer does the broadcast add (it was the
co-bottleneck at ~68us); it only drains PSUM->SBUF casts, with a slice
of drains moved to the scalar engine to balance. Positions are ordered
(u, t) so each tanh covers a full T row; the host transposes back.

b_out is added on the host during the gather.
"""

import numpy as np
import ml_dtypes

P = 128
B, T, U = 8, 200, 50
DE, DP, DJ, V = 512, 640, 640, 1024
NDE, NDP, NJC, NVC = DE // P, DP // P, DJ // P, V // P  # 4, 5, 5, 8
UB = 2               # u's per psum block
PBLK = UB * T        # 400 joint positions per psum tile
NUB = U // UB        # 25 u-groups
VQ = 4               # v-chunks ganged per SBUF out tile
NVH = NVC // VQ      # 2 v-halves

BF16 = ml_dtypes.bfloat16

_module = None


def _build_module():
    import concourse.bass as bass
    import concourse.mybir as mybir
    import concourse.tile as tile
    from concourse import bacc

    bf = mybir.dt.bfloat16
    f32 = mybir.dt.float32
    Alu = mybir.AluOpType
    Act = mybir.ActivationFunctionType
    ts, ds = bass.ts, bass.ds

    nc = bacc.Bacc("TRN2", target_bir_lowering=False, debug=False)

    d_encT = nc.dram_tensor("encT", (P, NDE, T), bf, kind="ExternalInput").ap()
    d_predT = nc.dram_tensor("predT", (P, NDP, U), bf, kind="ExternalInput").ap()
    d_wenc8 = nc.dram_tensor("wenc8", (P, NJC, NDE, P), bf, kind="ExternalInput").ap()
    d_wpred8 = nc.dram_tensor("wpred8", (P, NJC, NDP, P), bf, kind="ExternalInput").ap()
    d_woutT = nc.dram_tensor("woutT", (P, NJC, V), bf, kind="ExternalInput").ap()
    d_bj = nc.dram_tensor("bj", (P, NJC), f32, kind="ExternalInput").ap()
    d_out = nc.dram_tensor("out", (V, U * T), bf, kind="ExternalOutput").ap()

    with tile.TileContext(nc) as tc:
        with (
            tc.tile_pool(name="consts", bufs=1) as consts,
            tc.tile_pool(name="joints", bufs=4) as joints,
            tc.tile_pool(name="outsb", bufs=6) as outsb,
            tc.tile_pool(name="ps", bufs=8, space="PSUM") as pspool,
        ):
            # Input DMAs chunked per jc/vc in compute-priority order so the
            # projections chase the DMA stream (input DMA is ~9us at full
            # rate; all-at-once transfers stall the first matmul to ~17us).
            # Projection path on the sync ring; wout chunks on the (idle)
            # gpsimd ring so the scalar engine stays free for tanh.
            encT = consts.tile([P, NDE, T], bf)
            nc.sync.dma_start(encT[:], d_encT[:])
            predT = consts.tile([P, NDP, U], bf)
            nc.sync.dma_start(predT[:], d_predT[:])
            bj = consts.tile([P, NJC], f32)
            nc.sync.dma_start(bj[:], d_bj[:])
            # One DMA per tensor (each dma_start costs ~0.68us of SP issue
            # time, so chunked weight DMAs delay the last transfer), in
            # compute-priority order on a single FIFO ring.
            wenc8 = consts.tile([P, NJC, NDE, P], bf)
            nc.sync.dma_start(wenc8[:], d_wenc8[:])
            wpred8 = consts.tile([P, NJC, NDP, P], bf)
            nc.sync.dma_start(wpred8[:], d_wpred8[:])
            wout = consts.tile([P, NJC, V], bf)
            nc.sync.dma_start(wout[:], d_woutT[:])

            # --- projections -> encP[j, t] f32, predP[j, u] f32 with
            # (b_enc+b_pred) folded in.
            encP = consts.tile([P, NJC, T], f32)
            predP = consts.tile([P, NJC, U], f32)
            for jc in range(NJC):
                ps_e = pspool.tile([P, 512], f32, tag="ps")
                for dc in range(NDE):
                    nc.tensor.matmul(
                        ps_e[:, :T], wenc8[:, jc, dc, :], encT[:, dc, :],
                        start=(dc == 0), stop=(dc == NDE - 1),
                    )
                nc.vector.tensor_copy(encP[:, jc, :], ps_e[:, :T])

                ps_p = pspool.tile([P, 512], f32, tag="ps")
                for dc in range(NDP):
                    nc.tensor.matmul(
                        ps_p[:, :U], wpred8[:, jc, dc, :], predT[:, dc, :],
                        start=(dc == 0), stop=(dc == NDP - 1),
                    )
                nc.vector.tensor_tensor(
                    predP[:, jc, :], ps_p[:, :U],
                    bj[:, jc, None].to_broadcast((P, U)), Alu.add,
                )

            # --- main loop over u-groups (UB u's x full T per group)
            for ug in range(NUB):
                # one jt tile per group (not per jc): 1 semaphore wait on the
                # PE queue instead of 5, and 5x fewer teardown semaphores.
                jt = joints.tile([P, NJC, UB, T], bf, tag="jt")
                jtiles = []
                for jc in range(NJC):
                    for r in range(UB):
                        u = ug * UB + r
                        nc.scalar.activation(
                            jt[:, jc, r, :], encP[:, jc, :], Act.Tanh,
                            bias=predP[:, jc, u, None],
                        )
                    jtiles.append(jt[:, jc].rearrange("p a b -> p (a b)"))

                last = ug == NUB - 1
                for vh in range(NVH):
                    osb = outsb.tile([P, VQ, PBLK], bf, tag="osb")
                    for vq in range(VQ):
                        ps_o = pspool.tile([P, 512], f32, tag="ps")
                        for jc in range(NJC):
                            nc.tensor.matmul(
                                ps_o[:, :PBLK], wout[:, jc, ts(vh * VQ + vq, P)],
                                jtiles[jc],
                                start=(jc == 0), stop=(jc == NJC - 1),
                            )
                        # drains on vector (scalar stays tanh-only); for the
                        # final group alternate engines so the tail drains
                        # in parallel.
                        on_scalar = (vq & 1) if last else False
                        if on_scalar:
                            nc.scalar.copy(osb[:, vq, :], ps_o[:, :PBLK])
                        else:
                            nc.vector.tensor_copy(osb[:, vq, :], ps_o[:, :PBLK])
                    dst = (
                        d_out[ds(vh * VQ * P, VQ * P), ts(ug, PBLK)]
                        .rearrange("(q p) c -> p q c", p=P)
                    )
                    if last:
                        # split the final DMA across two rings to cut the tail
                        nc.sync.dma_start(dst[:, :2, :], osb[:, :2, :])
                        nc.scalar.dma_start(dst[:, 2:, :], osb[:, 2:, :])
                    else:
                        nc.sync.dma_start(dst, osb[:])

    nc.compile()
    return nc


def _get_module():
    global _module
    if _module is None:
        _module = _build_module()
    return _module


def _chunk(x2d, dtype=BF16):
    """(n*128, C...) -> (128, n, C...) partition-chunked, contiguous."""
    n = x2d.shape[0] // P
    return np.ascontiguousarray(
        x2d.reshape((n, P) + x2d.shape[1:]).swapaxes(0, 1)
    ).astype(dtype)


def _wchunk(W, n_out, n_in):
    """nn.Linear weight (out, in) -> (128_in, n_out, n_in, 128_out):
    [p, oc, ic, q] = W[oc*128+q, ic*128+p], per-jc/vc chunks contiguous."""
    A = W.reshape(n_out, P, n_in, P)        # (oc, q, ic, p)
    return np.ascontiguousarray(A.transpose(3, 0, 2, 1)).astype(BF16)


def make_in_maps(encoder_out, predictor_out, W_enc, b_enc, W_pred, b_pred, W_out, b_out):
    wenc8 = _wchunk(W_enc, NJC, NDE)        # (128, 5, 4, 128)
    wpred8 = _wchunk(W_pred, NJC, NDP)      # (128, 5, 5, 128)
    woutT = _chunk(np.ascontiguousarray(W_out.T))       # (128, 5, 1024)
    bj = np.ascontiguousarray(
        (b_enc + b_pred).reshape(NJC, P).T).astype(np.float32)   # (128, 5)
    in_maps = []
    for b in range(B):
        in_maps.append({
            "encT": _chunk(np.ascontiguousarray(encoder_out[b].T)),    # (128,4,200)
            "predT": _chunk(np.ascontiguousarray(predictor_out[b].T)), # (128,5,50)
            "wenc8": wenc8,
            "wpred8": wpred8,
            "woutT": woutT,
            "bj": bj,
        })
    return in_maps


def _postprocess(out_vt, b_out):
    """(V, U*T) device output (bf16, pos=(u,t)) -> (T, U, V) fp32 + bias."""
    arr = out_vt.astype(np.float32).T.reshape(U, T, V).swapaxes(0, 1)
    return arr + b_out.astype(np.float32)


def kernel(encoder_out, predictor_out, W_enc, b_enc, W_pred, b_pred, W_out, b_out):
    from concourse.bass_utils import run_bass_kernel_spmd

    nc = _get_module()
    in_maps = make_in_maps(
        encoder_out, predictor_out, W_enc, b_enc, W_pred, b_pred, W_out, b_out
    )
    res = run_bass_kernel_spmd(nc, in_maps, list(range(B)))
    out = np.empty((B, T, U, V), np.float32)
    for b in range(B):
        out[b] = _postprocess(res.results[b]["out"], b_out)
    return out
